# revision 20
# baseline (speedup 1.0000x reference)
"""Trainium2 Bass kernel for the NeuralMemory (scatter_memory) problem.

Math (B=1, N=512, D=128, DEPTH=4): per-token meta-gradients of the memory
MLP are rank-1 per layer, so the (n, depth, d, d) momentum/update scans
collapse to a scalar coefficient matrix C[t,s] applied attention-style:

    retrieved_l(t) = y_t @ W_l + sum_s C[t,s]*(-lr_s) * (y_t . x_l(s)) * g_l(s)

C is numerically banded (C[t,s]==0 in fp32 for t-s>=64), so each of the 8
cores handles one 64-query window with a 128-token key window -- fully
data-parallel, no collectives.  Core 0's missing past is zero-padded.

v3 latency notes (the kernel is latency-bound; every engine <55% busy):
  - Input DMA completion semaphores land ~1.9us after the DMA instruction
    retires (HBM read receipt round-trip).  The DMA is split into 4 chunks
    ordered by first use across BOTH HWDGE rings (SP + ACT), the first
    chunk minimal (seqW + fused wk0), and every DMA-gated matmul carries a
    tile_wait_until() hint so the Tile scheduler's static order matches
    hardware timing instead of its optimistic cost model.
  - h1 = (Wk@W0)^T @ seq^T via host-fused wk0 starts the chain at the
    first chunk's completion; x0 (keys^T) is computed off-chain later.
  - d4 = h4 - v^T forms inside one PSUM accumulation group (host-negated
    Wv start matmul runs early), fp16, so d4 needs only one DVE copy.
  - silu' chains: sp1/sp2 on GpSimd (4-op form), sp3 on DVE (3-op STT) so
    it beats b3 = wmT3 @ d4.  Backward deltas fp16 end-to-end.
  - The PE array is kept busy with dummy warm matmuls from ~4.0us so the
    HAM clock gate (1.2 -> 2.4 GHz after ~3.4us sustained activity) lifts
    before the backward/retrieval phases.
"""

import numpy as np

D = 128
N = 512
DEPTH = 4
NCORES = 8
QW = N // NCORES        # 64 queries per core
SW = 2 * QW             # 128-token key window per core

# column offsets inside the consolidated per-core input tensor (128, ALLW).
OFF_SEQW = 0                     # chunk 1 (SP ring) [0:256)
OFF_WK0 = 128                    # (Wk @ W_mem[0]) host-fused
OFF_WVN = 256                    # chunk 2 (SP ring) [256:768): -Wv
OFF_WM1 = 384
OFF_WM2 = 512
OFF_WM3 = 640
OFF_REPL = 768                   # chunk 3 (SP ring) [768:1536): reps first
OFF_REPM = 896
OFF_REPD = 1024
OFF_ID = 1152
OFF_WQ = 1280                    # wq/wk last: late sim-subtile readiness
OFF_WK = 1408
OFF_WMT = 1536                   # chunk 4 (ACT ring) [1536:2176): W^T x4
OFF_WM0 = 2048
ALLW = 2176

_cache = {}


def _build_program():
    import concourse.mybir as mybir
    from concourse import bacc
    from concourse.tile import TileContext

    f32 = mybir.dt.float32
    fp16 = mybir.dt.float16
    AF = mybir.ActivationFunctionType
    ALU = mybir.AluOpType

    nc = bacc.Bacc("TRN2")

    allin_d = nc.dram_tensor("allin", [D, ALLW], fp16, kind="ExternalInput")
    outT_d = nc.dram_tensor("outT", [D, QW], fp16, kind="ExternalOutput")

    with TileContext(nc) as tc:
        with (
            tc.tile_pool(name="sb", bufs=1) as sb,
            tc.tile_pool(name="ph", bufs=2, space="PSUM") as ph,  # chain+proj
            tc.tile_pool(name="pm", bufs=2, space="PSUM") as pm,  # d4,S*
            tc.tile_pool(name="pb", bufs=2, space="PSUM") as pb,  # b*,tr*
            tc.tile_pool(name="pr", bufs=1, space="PSUM") as pr,  # warm,r*
        ):
            def sbt(tag, shape=(D, SW), dt=fp16):
                return sb.tile(list(shape), dt, tag=tag, name=tag)

            allin = sbt("allin", (D, ALLW))
            # SP ring: 3 chunks ordered by first use; ACT ring: wmT chunk.
            nc.sync.dma_start(out=allin[:, 0:256], in_=allin_d[:, 0:256])
            nc.sync.dma_start(out=allin[:, 256:768], in_=allin_d[:, 256:768])
            nc.sync.dma_start(out=allin[:, 1536:2176],
                              in_=allin_d[:, 1536:2176])
            nc.sync.dma_start(out=allin[:, 768:1536], in_=allin_d[:, 768:1536])

            seqW = allin[:, OFF_SEQW:OFF_SEQW + SW]
            wk0 = allin[:, OFF_WK0:OFF_WK0 + D]
            wvn = allin[:, OFF_WVN:OFF_WVN + D]
            wm = [allin[:, OFF_WM0:OFF_WM0 + D],
                  allin[:, OFF_WM1:OFF_WM1 + D],
                  allin[:, OFF_WM2:OFF_WM2 + D],
                  allin[:, OFF_WM3:OFF_WM3 + D]]
            wq = allin[:, OFF_WQ:OFF_WQ + D]
            wk = allin[:, OFF_WK:OFF_WK + D]
            rep_lr = allin[:, OFF_REPL:OFF_REPL + D]
            rep_mom = allin[:, OFF_REPM:OFF_REPM + D]
            rep_dec = allin[:, OFF_REPD:OFF_REPD + D]
            idm = allin[:, OFF_ID:OFF_ID + D]
            wmT = [allin[:, OFF_WMT + D * l:OFF_WMT + D * (l + 1)]
                   for l in range(DEPTH)]

            # ---- PE clock warmup until chunk 1's completion (~6.1us) ----
            scrw = sbt("scrw", (D, D))
            nc.vector.memset(scrw, 0.0)
            warm = pr.tile([D, D], f32, tag="r", name="warm")
            for _ in range(11):
                nc.tensor.matmul(warm, scrw, scrw, start=True, stop=True)
            wsink = sbt("wsink", (D, 8))
            nc.vector.tensor_copy(wsink, warm[:, 0:8])

            # ---- forward chain.  Every DMA-gated projection matmul is
            # allocated from the SAME rotating PSUM pool as the chain's h
            # tiles, so slot WAR dependencies pin the static order (the
            # scheduler's optimistic DMA model can no longer hoist them
            # into the chain's path).
            ps_h1 = ph.tile([D, SW], f32, tag="h", name="h1")
            nc.tensor.matmul(ps_h1, wk0, seqW, start=True, stop=True)
            # d4 group: -v^T start matmul early, h4 accumulates later
            ps_d4 = pm.tile([D, SW], f32, tag="m", name="d4")
            nc.tensor.matmul(ps_d4, wvn, seqW, start=True, stop=False)
            x1 = sbt("x1")
            nc.scalar.activation(x1, ps_h1, AF.Silu)
            th1 = sbt("th1")
            nc.scalar.activation(th1, ps_h1, AF.Tanh, scale=0.5)

            ps_h2 = ph.tile([D, SW], f32, tag="h", name="h2")
            nc.tensor.matmul(ps_h2, wm[1], x1, start=True, stop=True)
            x2 = sbt("x2")
            nc.scalar.activation(x2, ps_h2, AF.Silu)
            th2 = sbt("th2")
            nc.scalar.activation(th2, ps_h2, AF.Tanh, scale=0.5)

            ps_h3 = ph.tile([D, SW], f32, tag="h", name="h3")
            nc.tensor.matmul(ps_h3, wm[2], x2, start=True, stop=True)
            th3 = sbt("th3")
            nc.scalar.activation(th3, ps_h3, AF.Tanh, scale=0.5)
            x3 = sbt("x3")
            nc.scalar.activation(x3, ps_h3, AF.Silu)

            # h4 accumulates onto -v^T: ps_d4 becomes d4 = h4 - v^T
            nc.tensor.matmul(ps_d4, wm[3], x3, start=False, stop=True)
            d4 = sbt("d4")
            nc.scalar.copy(d4, ps_d4)

            # sp1/sp2 on GpSimd (4-op form): s = 0.5+0.5*th; sp = s+x-x*s
            def sp_gp(th, x, pref):
                s = sbt(f"{pref}s")
                nc.gpsimd.tensor_scalar(s, th, 0.5, 0.5, ALU.mult, ALU.add)
                xs = sbt(f"{pref}xs")
                nc.gpsimd.tensor_mul(xs, x, s)
                u = sbt(f"{pref}u")
                nc.gpsimd.tensor_sub(u, x, xs)
                sp = sbt(f"{pref}sp")
                nc.gpsimd.tensor_add(sp, s, u)
                return sp

            sp1 = sp_gp(th1, x1, "s1")
            sp2 = sp_gp(th2, x2, "s2")

            # projections: slot-chained behind h2/h3 readers
            ps_lrb = ph.tile([D, SW], f32, tag="h", name="lrb")
            nc.tensor.matmul(ps_lrb, rep_lr, seqW, start=True, stop=True)
            ps_amb = ph.tile([D, SW], f32, tag="h", name="amb")
            nc.tensor.matmul(ps_amb, rep_mom, seqW, start=True, stop=True)

            # DVE: izlr, then sp3 back-to-back (same-engine, no sem gaps)
            izlr = sbt("izlr")
            nc.vector.tensor_mul(izlr, idm, ps_lrb)
            s3t = sbt("s3t")
            nc.vector.scalar_tensor_tensor(s3t, th3, -0.5, x3,
                                           ALU.mult, ALU.mult)
            s3w = sbt("s3w")
            nc.vector.scalar_tensor_tensor(s3w, th3, 1.0, x3,
                                           ALU.add, ALU.add)
            sp3 = sbt("sp3")
            nc.vector.scalar_tensor_tensor(sp3, s3w, 0.5, s3t,
                                           ALU.mult, ALU.add)
            AT = sbt("AT")
            nc.vector.tensor_tensor_scan(AT, ps_amb, izlr, 0.0,
                                         ALU.mult, ALU.add)

            # dec reuses lrb's slot (freed by izlr), q reuses dec's
            ps_dec = ph.tile([D, SW], f32, tag="h", name="dec")
            nc.tensor.matmul(ps_dec, rep_dec, seqW, start=True, stop=True)
            th_dec = sbt("th_dec")
            nc.scalar.activation(th_dec, ps_dec, AF.Tanh, scale=0.5)
            bb = sbt("bb")
            nc.gpsimd.tensor_scalar(bb, th_dec, -0.5, 0.5, ALU.mult, ALU.add)

            # ---- backward delta chain ----
            ps_b3 = pb.tile([D, SW], f32, tag="bt", name="b3")
            nc.tensor.matmul(ps_b3, wmT[3], d4, start=True, stop=True)
            d3 = sbt("d3")
            nc.vector.tensor_mul(d3, ps_b3, sp3)
            ps_b2 = pb.tile([D, SW], f32, tag="bt", name="b2")
            nc.tensor.matmul(ps_b2, wmT[2], d3, start=True, stop=True)
            d2 = sbt("d2")
            nc.vector.tensor_mul(d2, ps_b2, sp2)
            ps_b1 = pb.tile([D, SW], f32, tag="bt", name="b1")
            nc.tensor.matmul(ps_b1, wmT[1], d2, start=True, stop=True)
            d1 = sbt("d1")
            nc.vector.tensor_mul(d1, ps_b1, sp1)
            CT = sbt("CT")
            nc.vector.tensor_tensor_scan(CT, bb, AT, 0.0, ALU.mult, ALU.add)

            # x0/q: slot-chained behind amb/dec readers
            ps_x0 = ph.tile([D, SW], f32, tag="h", name="x0")
            nc.tensor.matmul(ps_x0, wk, seqW, start=True, stop=True)
            ps_q = ph.tile([D, QW], f32, tag="h", name="q")
            nc.tensor.matmul(ps_q, wq, seqW[:, QW:SW], start=True,
                             stop=True)

            # x0/qT staging on Scalar (DVE is saturated mid-kernel)
            x0 = sbt("x0")
            nc.scalar.copy(x0, ps_x0)
            qT = sbt("qT", (D, QW))
            nc.scalar.copy(qT, ps_q)

            Dl = {1: d1, 2: d2, 3: d3, 4: d4}
            X = [x0, x1, x2, x3]

            # ---- retrieval; G_l transposed lazily, most-critical first ----
            Y = qT
            CTq = CT[:, QW:SW]
            for l in range(DEPTH):
                ps_t = pb.tile([D, D], fp16, tag="bt", name=f"t{l}")
                nc.tensor.transpose(ps_t, Dl[l + 1], idm)
                gl = sbt(f"g{l}")
                nc.vector.tensor_copy(gl, ps_t)
                ps_s = pm.tile([D, QW], f32, tag="m", name=f"S{l}")
                nc.tensor.matmul(ps_s, X[l], Y, start=True, stop=True)
                cst = sbt(f"cst{l}", (D, QW))
                nc.vector.tensor_mul(cst, ps_s, CTq)
                ps_o = pr.tile([D, QW], f32, tag="r", name=f"r{l}")
                nc.tensor.matmul(ps_o, wm[l], Y, start=True, stop=False)
                nc.tensor.matmul(ps_o, gl, cst, start=False, stop=True)
                if l < DEPTH - 1:
                    ynext = sbt(f"y{l + 1}", (D, QW))
                    nc.scalar.activation(ynext, ps_o, AF.Silu)
                    Y = ynext
                else:
                    outT = sbt("outT", (D, QW), dt=fp16)
                    h = QW // 2
                    nc.vector.tensor_copy(outT[:, 0:h], ps_o[:, 0:h])
                    nc.sync.dma_start(out=outT_d[:, 0:h], in_=outT[:, 0:h])
                    nc.vector.tensor_copy(outT[:, h:QW], ps_o[:, h:QW])
                    nc.scalar.dma_start(out=outT_d[:, h:QW],
                                        in_=outT[:, h:QW])

    return nc


def get_program():
    if "nc" not in _cache:
        nc = _build_program()
        nc.finalize()
        _cache["nc"] = nc
    return _cache["nc"]


def make_in_maps(seq, W_mem, W_q, W_kv, W_mom, W_step, W_decay):
    seq = np.asarray(seq, dtype=np.float32)
    W_mem = np.asarray(W_mem, dtype=np.float32)
    W_kv = np.asarray(W_kv, dtype=np.float32)
    seqT = seq.reshape(N, D).T  # (d, n)

    base = np.zeros((D, ALLW), dtype=np.float16)
    base[:, OFF_WK0:OFF_WK0 + D] = W_kv[:, :D] @ W_mem[0]
    base[:, OFF_WVN:OFF_WVN + D] = -W_kv[:, D:]
    base[:, OFF_WK:OFF_WK + D] = W_kv[:, :D]
    base[:, OFF_WQ:OFF_WQ + D] = np.asarray(W_q, dtype=np.float32)
    for l in range(DEPTH):
        off = [OFF_WM0, OFF_WM1, OFF_WM2, OFF_WM3][l]
        base[:, off:off + D] = W_mem[l]
        base[:, OFF_WMT + D * l:OFF_WMT + D * (l + 1)] = W_mem[l].T
    lr_col = np.asarray(W_step, dtype=np.float32)[:, 0] * (-2.0 / D)
    base[:, OFF_REPL:OFF_REPL + D] = np.repeat(lr_col[:, None], D, axis=1)
    base[:, OFF_REPM:OFF_REPM + D] = np.repeat(
        np.asarray(W_mom, dtype=np.float32)[:, :1], D, axis=1)
    base[:, OFF_REPD:OFF_REPD + D] = np.repeat(
        np.asarray(W_decay, dtype=np.float32)[:, :1], D, axis=1)
    base[:, OFF_ID:OFF_ID + D] = np.eye(D, dtype=np.float32)

    in_maps = []
    for c in range(NCORES):
        allin = base.copy()
        qc = c * QW
        lo = qc - QW
        win = np.zeros((D, SW), dtype=np.float16)
        src_lo = max(lo, 0)
        win[:, src_lo - lo:] = seqT[:, src_lo:qc + QW].astype(np.float16)
        allin[:, OFF_SEQW:OFF_SEQW + SW] = win
        in_maps.append({"allin": allin})
    return in_maps


def assemble(results):
    out = np.empty((N, D), dtype=np.float32)
    for c in range(NCORES):
        out[c * QW:(c + 1) * QW, :] = results[c]["outT"].T.astype(np.float32)
    return out.reshape(1, N, D)


def kernel(**inputs) -> np.ndarray:
    from concourse.bass_utils import run_bass_kernel_spmd

    nc = get_program()
    in_maps = make_in_maps(**inputs)
    res = run_bass_kernel_spmd(nc, in_maps, list(range(NCORES)))
    return assemble(res.results)


# revision 21
# speedup vs baseline: 1.1209x; 1.1209x over previous
"""Trainium2 Bass kernel for the NeuralMemory (scatter_memory) problem.

Math (B=1, N=512, D=128, DEPTH=4): per-token meta-gradients of the memory
MLP are rank-1 per layer, so the (n, depth, d, d) momentum/update scans
collapse to a scalar coefficient matrix C[t,s] applied attention-style:

    retrieved_l(t) = y_t @ W_l + sum_s C[t,s]*(-lr_s) * (y_t . x_l(s)) * g_l(s)

C is numerically banded (C[t,s]==0 in fp32 for t-s>=64), so each of the 8
cores handles one 64-query window with a 128-token key window -- fully
data-parallel, no collectives.  Core 0's missing past is zero-padded.

v3 latency notes (the kernel is latency-bound; every engine <55% busy):
  - Input DMA completion semaphores land ~1.9us after the DMA instruction
    retires (HBM read receipt round-trip).  The DMA is split into 4 chunks
    ordered by first use across BOTH HWDGE rings (SP + ACT), the first
    chunk minimal (seqW + fused wk0), and every DMA-gated matmul carries a
    tile_wait_until() hint so the Tile scheduler's static order matches
    hardware timing instead of its optimistic cost model.
  - h1 = (Wk@W0)^T @ seq^T via host-fused wk0 starts the chain at the
    first chunk's completion; x0 (keys^T) is computed off-chain later.
  - d4 = h4 - v^T forms inside one PSUM accumulation group (host-negated
    Wv start matmul runs early), fp16, so d4 needs only one DVE copy.
  - silu' chains: sp1/sp2 on GpSimd (4-op form), sp3 on DVE (3-op STT) so
    it beats b3 = wmT3 @ d4.  Backward deltas fp16 end-to-end.
  - The PE array is kept busy with dummy warm matmuls from ~4.0us so the
    HAM clock gate (1.2 -> 2.4 GHz after ~3.4us sustained activity) lifts
    before the backward/retrieval phases.
"""

import numpy as np

D = 128
N = 512
DEPTH = 4
NCORES = 8
QW = N // NCORES        # 64 queries per core
SW = 2 * QW             # 128-token key window per core

# column offsets inside the consolidated per-core input tensor (128, ALLW).
OFF_SEQW = 0                     # chunk 1 (SP ring) [0:256)
OFF_WK0 = 128                    # (Wk @ W_mem[0]) host-fused
OFF_WVN = 256                    # chunk 2 (SP ring) [256:768): -Wv
OFF_WM1 = 384
OFF_WM2 = 512
OFF_WM3 = 640
OFF_REPL = 768                   # chunk 3 (SP ring) [768:1536): reps first
OFF_REPM = 896
OFF_REPD = 1024
OFF_ID = 1152
OFF_WQ = 1280                    # wq/wk last: late sim-subtile readiness
OFF_WK = 1408
OFF_WMT = 1536                   # chunk 4 (ACT ring) [1536:2176): W^T x4
OFF_WM0 = 2048
ALLW = 2176

_cache = {}


def _build_program():
    import concourse.mybir as mybir
    from concourse import bacc
    from concourse.tile import TileContext

    f32 = mybir.dt.float32
    fp16 = mybir.dt.float16
    AF = mybir.ActivationFunctionType
    ALU = mybir.AluOpType

    nc = bacc.Bacc("TRN2")

    allin_d = nc.dram_tensor("allin", [D, ALLW], fp16, kind="ExternalInput")
    outT_d = nc.dram_tensor("outT", [D, QW], fp16, kind="ExternalOutput")

    with TileContext(nc) as tc:
        with (
            tc.tile_pool(name="sb", bufs=1) as sb,
            tc.tile_pool(name="ph", bufs=2, space="PSUM") as ph,  # chain+proj
            tc.tile_pool(name="pm", bufs=2, space="PSUM") as pm,  # d4,S*
            tc.tile_pool(name="pb", bufs=2, space="PSUM") as pb,  # b*,tr*
            tc.tile_pool(name="pr", bufs=1, space="PSUM") as pr,  # warm,r*
        ):
            def sbt(tag, shape=(D, SW), dt=fp16):
                return sb.tile(list(shape), dt, tag=tag, name=tag)

            allin = sbt("allin", (D, ALLW))
            # SP ring: 3 chunks ordered by first use; ACT ring: wmT chunk.
            nc.sync.dma_start(out=allin[:, 0:256], in_=allin_d[:, 0:256])
            nc.sync.dma_start(out=allin[:, 256:768], in_=allin_d[:, 256:768])
            nc.sync.dma_start(out=allin[:, 1536:2176],
                              in_=allin_d[:, 1536:2176])
            nc.sync.dma_start(out=allin[:, 768:1536], in_=allin_d[:, 768:1536])

            seqW = allin[:, OFF_SEQW:OFF_SEQW + SW]
            wk0 = allin[:, OFF_WK0:OFF_WK0 + D]
            wvn = allin[:, OFF_WVN:OFF_WVN + D]
            wm = [allin[:, OFF_WM0:OFF_WM0 + D],
                  allin[:, OFF_WM1:OFF_WM1 + D],
                  allin[:, OFF_WM2:OFF_WM2 + D],
                  allin[:, OFF_WM3:OFF_WM3 + D]]
            wq = allin[:, OFF_WQ:OFF_WQ + D]
            wk = allin[:, OFF_WK:OFF_WK + D]
            rep_lr = allin[:, OFF_REPL:OFF_REPL + D]
            rep_mom = allin[:, OFF_REPM:OFF_REPM + D]
            rep_dec = allin[:, OFF_REPD:OFF_REPD + D]
            idm = allin[:, OFF_ID:OFF_ID + D]
            wmT = [allin[:, OFF_WMT + D * l:OFF_WMT + D * (l + 1)]
                   for l in range(DEPTH)]

            # ---- PE clock warmup until chunk 1's completion (~6.1us) ----
            scrw = sbt("scrw", (D, D))
            nc.vector.memset(scrw, 0.0)
            warm = pr.tile([D, D], f32, tag="r", name="warm")
            for _ in range(11):
                nc.tensor.matmul(warm, scrw, scrw, start=True, stop=True)
            wsink = sbt("wsink", (D, 8))
            nc.vector.tensor_copy(wsink, warm[:, 0:8])

            # ---- forward chain.  Every DMA-gated projection matmul is
            # allocated from the SAME rotating PSUM pool as the chain's h
            # tiles, so slot WAR dependencies pin the static order (the
            # scheduler's optimistic DMA model can no longer hoist them
            # into the chain's path).
            ps_h1 = ph.tile([D, SW], f32, tag="h", name="h1")
            nc.tensor.matmul(ps_h1, wk0, seqW, start=True, stop=True)
            # d4 group: -v^T start matmul early, h4 accumulates later
            ps_d4 = pm.tile([D, SW], f32, tag="m", name="d4")
            nc.tensor.matmul(ps_d4, wvn, seqW, start=True, stop=False)
            x1 = sbt("x1")
            nc.scalar.activation(x1, ps_h1, AF.Silu)
            th1 = sbt("th1")
            nc.scalar.activation(th1, ps_h1, AF.Tanh, scale=0.5)

            ps_h2 = ph.tile([D, SW], f32, tag="h", name="h2")
            nc.tensor.matmul(ps_h2, wm[1], x1, start=True, stop=True)
            x2 = sbt("x2")
            nc.scalar.activation(x2, ps_h2, AF.Silu)
            th2 = sbt("th2")
            nc.scalar.activation(th2, ps_h2, AF.Tanh, scale=0.5)

            ps_h3 = ph.tile([D, SW], f32, tag="h", name="h3")
            nc.tensor.matmul(ps_h3, wm[2], x2, start=True, stop=True)
            x3 = sbt("x3")
            nc.scalar.activation(x3, ps_h3, AF.Silu)
            th3 = sbt("th3")
            nc.scalar.activation(th3, ps_h3, AF.Tanh, scale=0.5)

            # h4 accumulates onto -v^T: ps_d4 becomes d4 = h4 - v^T
            nc.tensor.matmul(ps_d4, wm[3], x3, start=False, stop=True)
            d4 = sbt("d4")
            nc.scalar.copy(d4, ps_d4)

            # sp1/sp2 on GpSimd (4-op form): s = 0.5+0.5*th; sp = s+x-x*s
            def sp_gp(th, x, pref):
                s = sbt(f"{pref}s")
                nc.gpsimd.tensor_scalar(s, th, 0.5, 0.5, ALU.mult, ALU.add)
                xs = sbt(f"{pref}xs")
                nc.gpsimd.tensor_mul(xs, x, s)
                u = sbt(f"{pref}u")
                nc.gpsimd.tensor_sub(u, x, xs)
                sp = sbt(f"{pref}sp")
                nc.gpsimd.tensor_add(sp, s, u)
                return sp

            sp1 = sp_gp(th1, x1, "s1")
            sp2 = sp_gp(th2, x2, "s2")

            # projections: slot-chained behind h2/h3 readers; amb goes
            # behind izlr (lrb's reader) so the AT scan can't jump the
            # sp3 chain in the static DVE order
            ps_lrb = ph.tile([D, SW], f32, tag="h", name="lrb")
            nc.tensor.matmul(ps_lrb, rep_lr, seqW, start=True, stop=True)
            ps_dec = ph.tile([D, SW], f32, tag="h", name="dec")
            nc.tensor.matmul(ps_dec, rep_dec, seqW, start=True, stop=True)

            # DVE: izlr, then sp3 back-to-back (same-engine, no sem gaps)
            izlr = sbt("izlr")
            nc.vector.tensor_mul(izlr, idm, ps_lrb)
            s3t = sbt("s3t")
            nc.vector.scalar_tensor_tensor(s3t, th3, -0.5, x3,
                                           ALU.mult, ALU.mult)
            s3w = sbt("s3w")
            nc.vector.scalar_tensor_tensor(s3w, th3, 1.0, x3,
                                           ALU.add, ALU.add)
            sp3 = sbt("sp3")
            nc.vector.scalar_tensor_tensor(sp3, s3w, 0.5, s3t,
                                           ALU.mult, ALU.add)

            ps_amb = ph.tile([D, SW], f32, tag="h", name="amb")
            nc.tensor.matmul(ps_amb, rep_mom, seqW, start=True, stop=True)
            AT = sbt("AT")
            nc.vector.tensor_tensor_scan(AT, ps_amb, izlr, 0.0,
                                         ALU.mult, ALU.add)
            th_dec = sbt("th_dec")
            nc.scalar.activation(th_dec, ps_dec, AF.Tanh, scale=0.5)
            bb = sbt("bb")
            nc.gpsimd.tensor_scalar(bb, th_dec, -0.5, 0.5, ALU.mult, ALU.add)

            # ---- backward delta chain ----
            ps_b3 = pb.tile([D, SW], f32, tag="bt", name="b3")
            nc.tensor.matmul(ps_b3, wmT[3], d4, start=True, stop=True)
            d3 = sbt("d3")
            nc.vector.tensor_mul(d3, ps_b3, sp3)
            ps_b2 = pb.tile([D, SW], f32, tag="bt", name="b2")
            nc.tensor.matmul(ps_b2, wmT[2], d3, start=True, stop=True)
            d2 = sbt("d2")
            nc.vector.tensor_mul(d2, ps_b2, sp2)
            ps_b1 = pb.tile([D, SW], f32, tag="bt", name="b1")
            nc.tensor.matmul(ps_b1, wmT[1], d2, start=True, stop=True)
            d1 = sbt("d1")
            nc.vector.tensor_mul(d1, ps_b1, sp1)
            CT = sbt("CT")
            nc.vector.tensor_tensor_scan(CT, bb, AT, 0.0, ALU.mult, ALU.add)

            # x0/q: slot-chained behind amb/dec readers
            ps_x0 = ph.tile([D, SW], f32, tag="h", name="x0")
            nc.tensor.matmul(ps_x0, wk, seqW, start=True, stop=True)
            ps_q = ph.tile([D, QW], f32, tag="h", name="q")
            nc.tensor.matmul(ps_q, wq, seqW[:, QW:SW], start=True,
                             stop=True)

            # x0/qT staging on Scalar (DVE is saturated mid-kernel)
            x0 = sbt("x0")
            nc.scalar.copy(x0, ps_x0)
            qT = sbt("qT", (D, QW))
            nc.scalar.copy(qT, ps_q)

            Dl = {1: d1, 2: d2, 3: d3, 4: d4}
            X = [x0, x1, x2, x3]

            # ---- retrieval; G_l transposed lazily, most-critical first ----
            Y = qT
            CTq = CT[:, QW:SW]
            for l in range(DEPTH):
                ps_t = pb.tile([D, D], fp16, tag="bt", name=f"t{l}")
                nc.tensor.transpose(ps_t, Dl[l + 1], idm)
                gl = sbt(f"g{l}")
                nc.vector.tensor_copy(gl, ps_t)
                ps_s = pm.tile([D, QW], f32, tag="m", name=f"S{l}")
                nc.tensor.matmul(ps_s, X[l], Y, start=True, stop=True)
                cst = sbt(f"cst{l}", (D, QW))
                nc.vector.tensor_mul(cst, ps_s, CTq)
                ps_o = pr.tile([D, QW], f32, tag="r", name=f"r{l}")
                nc.tensor.matmul(ps_o, wm[l], Y, start=True, stop=False)
                nc.tensor.matmul(ps_o, gl, cst, start=False, stop=True)
                if l < DEPTH - 1:
                    ynext = sbt(f"y{l + 1}", (D, QW))
                    nc.scalar.activation(ynext, ps_o, AF.Silu)
                    Y = ynext
                else:
                    outT = sbt("outT", (D, QW), dt=fp16)
                    h = QW // 2
                    nc.vector.tensor_copy(outT[:, 0:h], ps_o[:, 0:h])
                    nc.sync.dma_start(out=outT_d[:, 0:h], in_=outT[:, 0:h])
                    nc.vector.tensor_copy(outT[:, h:QW], ps_o[:, h:QW])
                    nc.scalar.dma_start(out=outT_d[:, h:QW],
                                        in_=outT[:, h:QW])

    return nc


def get_program():
    if "nc" not in _cache:
        nc = _build_program()
        nc.finalize()
        _cache["nc"] = nc
    return _cache["nc"]


def make_in_maps(seq, W_mem, W_q, W_kv, W_mom, W_step, W_decay):
    seq = np.asarray(seq, dtype=np.float32)
    W_mem = np.asarray(W_mem, dtype=np.float32)
    W_kv = np.asarray(W_kv, dtype=np.float32)
    seqT = seq.reshape(N, D).T  # (d, n)

    base = np.zeros((D, ALLW), dtype=np.float16)
    base[:, OFF_WK0:OFF_WK0 + D] = W_kv[:, :D] @ W_mem[0]
    base[:, OFF_WVN:OFF_WVN + D] = -W_kv[:, D:]
    base[:, OFF_WK:OFF_WK + D] = W_kv[:, :D]
    base[:, OFF_WQ:OFF_WQ + D] = np.asarray(W_q, dtype=np.float32)
    for l in range(DEPTH):
        off = [OFF_WM0, OFF_WM1, OFF_WM2, OFF_WM3][l]
        base[:, off:off + D] = W_mem[l]
        base[:, OFF_WMT + D * l:OFF_WMT + D * (l + 1)] = W_mem[l].T
    lr_col = np.asarray(W_step, dtype=np.float32)[:, 0] * (-2.0 / D)
    base[:, OFF_REPL:OFF_REPL + D] = np.repeat(lr_col[:, None], D, axis=1)
    base[:, OFF_REPM:OFF_REPM + D] = np.repeat(
        np.asarray(W_mom, dtype=np.float32)[:, :1], D, axis=1)
    base[:, OFF_REPD:OFF_REPD + D] = np.repeat(
        np.asarray(W_decay, dtype=np.float32)[:, :1], D, axis=1)
    base[:, OFF_ID:OFF_ID + D] = np.eye(D, dtype=np.float32)

    in_maps = []
    for c in range(NCORES):
        allin = base.copy()
        qc = c * QW
        lo = qc - QW
        win = np.zeros((D, SW), dtype=np.float16)
        src_lo = max(lo, 0)
        win[:, src_lo - lo:] = seqT[:, src_lo:qc + QW].astype(np.float16)
        allin[:, OFF_SEQW:OFF_SEQW + SW] = win
        in_maps.append({"allin": allin})
    return in_maps


def assemble(results):
    out = np.empty((N, D), dtype=np.float32)
    for c in range(NCORES):
        out[c * QW:(c + 1) * QW, :] = results[c]["outT"].T.astype(np.float32)
    return out.reshape(1, N, D)


def kernel(**inputs) -> np.ndarray:
    from concourse.bass_utils import run_bass_kernel_spmd

    nc = get_program()
    in_maps = make_in_maps(**inputs)
    res = run_bass_kernel_spmd(nc, in_maps, list(range(NCORES)))
    return assemble(res.results)
